# revision 7
# baseline (speedup 1.0000x reference)
"""Nested-PPGN forward on 8 Trainium2 cores.

Strategy: data-parallel over subgraphs. The local PPGN stage shards the 480
subgraphs 60/core; the global stage shards the 16 graphs 2/core. Host does
only index preprocessing / layout (edge scatter into dense form, embedding
gathers, diag-embed); all matmul/conv/einsum/pool compute runs on device.

Per-core Bass kernel (same program for both stages, parameterized):
  zz0 [C0, NBCORE*NN*NN] (f32r) -> 2x regular blocks -> diag/offdiag maxpool
  -> fc1(relu) -> fc2 -> out [NOUT, NBCORE]
Each regular block: m1 = mlp(zz), m2 = mlp(zz) (1x1 convs on the tensor
engine, channels on partitions), mult = per-channel NxN matmul computed on
the vector engine via broadcast-AP product + X-reduction, then skip conv
accumulating the zz-part and mult-part into one PSUM group.
"""
import numpy as np
from contextlib import ExitStack

import concourse.bass as bass
import concourse.bacc as bacc
import concourse.tile as tile
from concourse import mybir
from concourse import bass_utils

F32 = mybir.dt.float32
F32R = mybir.dt.float32r
AX = mybir.AxisListType
ALU = mybir.AluOpType
ACTF = mybir.ActivationFunctionType

N_CORES = 8

# local stage: 480 subgraphs of 25 nodes, 60/core, groups of 4 (2500 px)
# global stage: 16 graphs of 30 nodes, 2/core, one group of 2 (1800 px)
LOCAL_CFG = dict(name="local", nn=25, c0=25, nb_core=60, ns=4, cp=500, nout=16)
GLOBAL_CFG = dict(name="global", nn=30, c0=22, nb_core=2, ns=2, cp=450, nout=1)

MASK_NEG = -1.0e9


def _flat(t):
    """[P, ...] contiguous tile -> [P, free_size] view."""
    a = t[:] if hasattr(t, "ap") is False else t
    n = 1
    for d in a.ap[1:]:
        n *= d[1]
    return bass.AP(tensor=a.tensor, offset=a.offset,
                   ap=[list(a.ap[0]), [1, n]])


def _ap(t, extra_dims, offset=0):
    """Manual AP over a tile: keep partition dim, replace free dims."""
    return bass.AP(tensor=t.tensor, offset=t.offset + offset,
                   ap=[list(t.ap[0])] + [list(d) for d in extra_dims])


def build_stage(cfg):
    nn = cfg["nn"]; c0 = cfg["c0"]; nbc = cfg["nb_core"]
    ns = cfg["ns"]; cp = cfg["cp"]; nout = cfg["nout"]
    px = nn * nn
    gpx = ns * px                  # pixels per group
    nchunk = gpx // cp             # matmul chunks per conv pass
    assert nchunk * cp == gpx
    ngroup = nbc // ns
    nkc = nn // 5                  # k-chunks of 5 for the mult
    assert nkc * 5 == nn

    nc = bacc.Bacc("TRN2", target_bir_lowering=False, debug=False,
                   enable_asserts=True, num_devices=N_CORES)

    zz0_d = nc.dram_tensor("zz0", [c0, nbc * px], F32R, kind="ExternalInput").ap()
    wk0_d = nc.dram_tensor("wk0", [c0, 3 * 128], F32R, kind="ExternalInput").ap()
    wk1_d = nc.dram_tensor("wk1", [128, 11 * 128 + nout], F32R, kind="ExternalInput").ap()
    bias_d = nc.dram_tensor("biases", [128, 12], F32, kind="ExternalInput").ap()
    mask_d = nc.dram_tensor("mask", [128, gpx], F32, kind="ExternalInput").ap()
    out_d = nc.dram_tensor("out", [nout, nbc], F32, kind="ExternalOutput").ap()

    with tile.TileContext(nc) as tc, ExitStack() as ctx:
        singles = ctx.enter_context(tc.tile_pool(name="singles", bufs=1))
        zz0p = ctx.enter_context(tc.tile_pool(name="zz0p", bufs=2))
        hp = ctx.enter_context(tc.tile_pool(name="hp", bufs=2))
        m1p = ctx.enter_context(tc.tile_pool(name="m1p", bufs=2))
        m2p = ctx.enter_context(tc.tile_pool(name="m2p", bufs=2))
        prodp = ctx.enter_context(tc.tile_pool(name="prodp", bufs=2))
        multp = ctx.enter_context(tc.tile_pool(name="multp", bufs=1))
        zz1p = ctx.enter_context(tc.tile_pool(name="zz1p", bufs=1))
        zz2p = ctx.enter_context(tc.tile_pool(name="zz2p", bufs=1))
        psum = ctx.enter_context(tc.tile_pool(name="psum", bufs=5, space="PSUM"))
        psfc = ctx.enter_context(tc.tile_pool(name="psfc", bufs=1, space="PSUM"))

        wk0 = singles.tile([c0, 3 * 128], F32R)
        wk1 = singles.tile([128, 11 * 128 + nout], F32R)
        biases = singles.tile([128, 12], F32)
        mask = singles.tile([128, gpx], F32)
        nc.sync.dma_start(out=wk0, in_=wk0_d)
        nc.sync.dma_start(out=wk1, in_=wk1_d)
        nc.sync.dma_start(out=biases, in_=bias_d)
        nc.sync.dma_start(out=mask, in_=mask_d)

        dpool = singles.tile([128, nbc], F32R)
        opool = singles.tile([128, nbc], F32R)

        def w0(j):  # [c0,128] stationary
            return wk0[:, 128 * j:128 * (j + 1)]

        def w1(j):  # [128,128] stationary
            return wk1[:, 128 * j:128 * (j + 1)]

        def bias(j):
            return biases[:, j:j + 1]

        def conv(out_t, srcs, bj, relu):
            """1x1 conv: out[128, gpx] = act(sum_i wT_i.T @ rhs_i + bias)."""
            of = _flat(out_t)
            for cch in range(nchunk):
                ps = psum.tile([128, cp], F32)
                for i, (wT, rhs) in enumerate(srcs):
                    nc.tensor.matmul(ps, wT, rhs[:, cch * cp:(cch + 1) * cp],
                                     start=(i == 0), stop=(i == len(srcs) - 1))
                dst = of[:, cch * cp:(cch + 1) * cp]
                if relu:
                    nc.scalar.activation(dst, ps, ACTF.Relu, bias=bias(bj))
                else:
                    nc.vector.tensor_scalar_add(dst, ps, bias(bj))

        def mlp(out_t, zz_f, w_l1, w_l2, bj1, bj2):
            h = hp.tile([128, gpx], F32R, tag="h")
            conv(h, [(w_l1, zz_f)], bj1, True)
            conv(out_t, [(w_l2, _flat(h))], bj2, True)

        def mult_stage(mult_t, m1, m2):
            # mult[o,k,m] = sum_n m1[o,k,n]*m2[o,n,m] on the vector engine
            for s in range(ns):
                for q in range(nkc):
                    prod = prodp.tile([128, 5, nn, nn], F32, tag="prod")
                    in0 = m1[:, s, 5 * q:5 * q + 5, :].unsqueeze(2).to_broadcast(
                        [128, 5, nn, nn])
                    in1 = m2[:, s, :, :].unsqueeze(1).transpose(
                        [0, 1, 3, 2]).to_broadcast([128, 5, nn, nn])
                    nc.vector.tensor_tensor(out=prod, in0=in0, in1=in1,
                                            op=ALU.mult)
                    with nc.allow_low_precision(reason="f32r is fp32-width"):
                        nc.vector.tensor_reduce(
                            out=mult_t[:, s, 5 * q:5 * q + 5, :],
                            in_=prod, axis=AX.X, op=ALU.add)

        def block(zz_f, zz_k, out_t, w_m1l1, w_m1l2, w_m2l1, w_m2l2,
                  w_sk_a, w_sk_b, bj0, relu_out=False):
            # zz_f: flat [c_in, gpx] f32r AP; zz_k: K of zz (c0 or 128)
            m1 = m1p.tile([128, ns, nn, nn], F32R, tag="m1")
            m2 = m2p.tile([128, ns, nn, nn], F32R, tag="m2")
            mlp(m1, zz_f, w_m1l1, w_m1l2, bj0 + 0, bj0 + 1)
            mlp(m2, zz_f, w_m2l1, w_m2l2, bj0 + 2, bj0 + 3)
            multb = multp.tile([128, ns, nn, nn], F32R, tag="multb")
            mult_stage(multb, m1, m2)
            conv(out_t, [(w_sk_a, zz_f), (w_sk_b, _flat(multb))],
                 bj0 + 4, relu_out)

        for g in range(ngroup):
            zz0g = zz0p.tile([c0, gpx], F32R, tag="zz0g")
            nc.sync.dma_start(out=zz0g, in_=zz0_d[:, g * gpx:(g + 1) * gpx])

            zz1 = zz1p.tile([128, ns, nn, nn], F32R, tag="zz1")
            block(_flat(zz0g), c0, zz1,
                  w0(0), w1(0), w0(1), w1(1), w0(2), w1(2), 0)
            zz2 = zz2p.tile([128, ns, nn, nn], F32, tag="zz2")
            block(_flat(zz1), 128, zz2,
                  w1(3), w1(4), w1(5), w1(6), w1(7), w1(8), 5)

            # diag / offdiag max-pool
            diag_ap = _ap(zz2, [[px, ns], [nn + 1, nn]])
            nc.vector.tensor_reduce(out=dpool[:, g * ns:(g + 1) * ns],
                                    in_=diag_ap, axis=AX.X, op=ALU.max)
            masked = prodp.tile([128, gpx], F32, tag="prod")
            nc.vector.tensor_tensor(out=masked, in0=_flat(zz2), in1=mask,
                                    op=ALU.add)
            off_ap = _ap(masked, [[px, ns], [1, px]])
            nc.vector.tensor_reduce(out=opool[:, g * ns:(g + 1) * ns],
                                    in_=off_ap, axis=AX.X, op=ALU.max)

        # fc head
        ps1 = psfc.tile([128, nbc], F32)
        nc.tensor.matmul(ps1, w1(9), dpool, start=True, stop=False)
        nc.tensor.matmul(ps1, w1(10), opool, start=False, stop=True)
        zl1 = singles.tile([128, nbc], F32R)
        nc.scalar.activation(zl1, ps1, ACTF.Relu, bias=bias(10))
        ps2 = psfc.tile([nout, nbc], F32)
        nc.tensor.matmul(ps2, wk1[:, 11 * 128:11 * 128 + nout], zl1,
                         start=True, stop=True)
        outt = singles.tile([nout, nbc], F32)
        nc.vector.tensor_scalar_add(outt, ps2, biases[:nout, 11:12])
        nc.sync.dma_start(out=out_d, in_=outt)

    nc.compile()
    return nc


_NC_CACHE = {}


def _get_nc(key):
    if key not in _NC_CACHE:
        cfg = LOCAL_CFG if key == "local" else GLOBAL_CFG
        _NC_CACHE[key] = build_stage(cfg)
    return _NC_CACHE[key]


# ---------------- host-side preprocessing ----------------

def _np(a):
    return np.asarray(a)


def build_zz(node_feat, pos, edge_index, edge_attr4, nb, nn):
    e = edge_attr4.shape[0]
    ea = np.concatenate([_np(edge_attr4).astype(np.float32),
                         np.ones((e, 1), np.float32)], 1)
    row = _np(edge_index[0]).astype(np.int64)
    col = _np(edge_index[1]).astype(np.int64)
    b, r, c = row // nn, row % nn, col % nn
    dense = np.zeros((nb, nn, nn, 5), np.float32)
    np.add.at(dense, (b, r, c), ea)
    dp = _np(pos).astype(np.float32).reshape(nb, nn, 3)
    d2 = ((dp[:, :, None, :] - dp[:, None, :, :]) ** 2).sum(-1)
    dmat = np.where(d2 > 0, np.sqrt(np.where(d2 > 0, d2, 1.0)), 0.0)
    dmat = dmat.astype(np.float32)
    dn = _np(node_feat).astype(np.float32).reshape(nb, nn, -1)
    eye = np.eye(nn, dtype=np.float32)
    diag = dn[:, :, None, :] * eye[None, :, :, None]
    zz_pre = np.concatenate([dense, dmat[..., None], diag], -1)
    zz = np.transpose(zz_pre, (0, 3, 2, 1))        # [nb, C, nn, nn]
    return np.ascontiguousarray(zz.reshape(nb, zz.shape[1], nn * nn))


def pack_weights(blocks, fc1, fc2, c0, nout):
    """Build wk0 [c0, 384], wk1 [128, 11*128+nout], biases [128, 12]."""
    def WT(wb):
        return np.ascontiguousarray(_np(wb[0]).astype(np.float32).T)

    def B(wb):
        return _np(wb[1]).astype(np.float32)

    b1, b2 = blocks
    wk0 = np.concatenate([WT(b1["mlp1"][0]), WT(b1["mlp2"][0]),
                          WT(b1["skip"])[:c0]], axis=1)
    sk1 = WT(b1["skip"])           # [c0+128, 128]
    sk2 = WT(b2["skip"])           # [256, 128]
    fc1T = WT(fc1)                 # [256, 128]
    w1_parts = [WT(b1["mlp1"][1]), WT(b1["mlp2"][1]), sk1[c0:],
                WT(b2["mlp1"][0]), WT(b2["mlp1"][1]),
                WT(b2["mlp2"][0]), WT(b2["mlp2"][1]),
                sk2[:128], sk2[128:], fc1T[:128], fc1T[128:],
                WT(fc2)]           # last: [128, nout]
    wk1 = np.concatenate(w1_parts, axis=1)
    biases = np.zeros((128, 12), np.float32)
    cols = [B(b1["mlp1"][0]), B(b1["mlp1"][1]), B(b1["mlp2"][0]),
            B(b1["mlp2"][1]), B(b1["skip"]), B(b2["mlp1"][0]),
            B(b2["mlp1"][1]), B(b2["mlp2"][0]), B(b2["mlp2"][1]),
            B(b2["skip"]), B(fc1)]
    for j, v in enumerate(cols):
        biases[:len(v), j] = v
    fb = B(fc2)
    biases[:len(fb), 11] = fb
    return wk0, wk1, biases


def make_mask(nn, ns):
    m = np.zeros((128, ns, nn, nn), np.float32)
    idx = np.arange(nn)
    m[:, :, idx, idx] = MASK_NEG
    return np.ascontiguousarray(m.reshape(128, ns * nn * nn))


LAST_PERF = {}
_RUNNER_CACHE = {}


def _get_runner(key):
    """Build-once jitted SPMD executor for a stage (mirrors
    bass2jax.run_bass_via_pjrt but caches the jitted callable)."""
    if key in _RUNNER_CACHE:
        return _RUNNER_CACHE[key]
    import jax
    from jax.experimental.shard_map import shard_map
    from jax.sharding import Mesh, PartitionSpec
    from concourse import bass2jax
    from concourse.bass2jax import _bass_exec_p, partition_id_tensor

    nc = _get_nc(key)
    bass2jax.install_neuronx_cc_hook()
    partition_name = nc.partition_id_tensor.name if nc.partition_id_tensor else None
    in_names, out_names, out_avals = [], [], []
    for alloc in nc.m.functions[0].allocations:
        if not isinstance(alloc, mybir.MemoryLocationSet):
            continue
        name = alloc.memorylocations[0].name
        if alloc.kind == "ExternalInput":
            if name != partition_name:
                in_names.append(name)
        elif alloc.kind == "ExternalOutput":
            out_names.append(name)
            out_avals.append(jax.core.ShapedArray(
                tuple(alloc.tensor_shape), mybir.dt.np(alloc.dtype)))
    n_params = len(in_names)
    all_names = list(in_names) + list(out_names)
    if partition_name is not None:
        all_names.append(partition_name)
    out_shapes = [(a.shape, a.dtype) for a in out_avals]

    def _body(*args):
        operands = list(args)
        if partition_name is not None:
            operands.append(partition_id_tensor())
        outs = _bass_exec_p.bind(
            *operands, out_avals=tuple(out_avals), in_names=tuple(all_names),
            out_names=tuple(out_names), lowering_input_output_aliases=(),
            sim_require_finite=True, sim_require_nnan=True, nc=nc)
        return tuple(outs)

    n_outs = len(out_avals)
    devices = jax.devices()[:N_CORES]
    mesh = Mesh(np.asarray(devices), ("core",))
    in_specs = (PartitionSpec("core"),) * (n_params + n_outs)
    out_specs = (PartitionSpec("core"),) * n_outs
    donate = tuple(range(n_params, n_params + n_outs))
    sharded = jax.jit(
        shard_map(_body, mesh=mesh, in_specs=in_specs, out_specs=out_specs,
                  check_rep=False),
        donate_argnums=donate, keep_unused=True)

    def run(in_maps):
        concat_in = [np.concatenate([m[nm] for m in in_maps], axis=0)
                     for nm in in_names]
        concat_zeros = [np.zeros((N_CORES * s[0],) + tuple(s[1:]), dt)
                        for (s, dt) in out_shapes]
        out_arrs = sharded(*concat_in, *concat_zeros)
        return [
            {nm: np.asarray(out_arrs[i]).reshape(
                (N_CORES,) + tuple(out_shapes[i][0]))[c]
             for i, nm in enumerate(out_names)}
            for c in range(N_CORES)
        ]

    _RUNNER_CACHE[key] = run
    return run


def run_stage(key, cfg, zz, wk0, wk1, biases):
    nbc = cfg["nb_core"]; nn = cfg["nn"]
    mask = make_mask(nn, cfg["ns"])
    in_maps = []
    for core in range(N_CORES):
        zzc = zz[core * nbc:(core + 1) * nbc]          # [nbc, C, nn*nn]
        zz0 = np.ascontiguousarray(
            np.transpose(zzc, (1, 0, 2)).reshape(zzc.shape[1], nbc * nn * nn))
        in_maps.append({"zz0": zz0, "wk0": wk0, "wk1": wk1,
                        "biases": biases, "mask": mask})
    import os
    if os.environ.get("KERNEL_FAST_RUNNER", "0") == "1":
        run = _get_runner(key)
        results = run(in_maps)
        return np.concatenate([r["out"].T for r in results], axis=0)
    nc = _get_nc(key)
    res = bass_utils.run_bass_kernel_spmd(nc, in_maps,
                                          core_ids=list(range(N_CORES)))
    return np.concatenate([r["out"].T for r in res.results], axis=0)


def kernel(x, pos, edge_attr, original_edge_attr, original_pos, params,
           z, node_type, edge_index, node_to_subgraph,
           original_edge_index, subgraph_to_graph):
    p = params
    h = _np(p["nt_emb"]).astype(np.float32)[_np(node_type).astype(np.int64)] \
        + _np(p["z_emb"]).astype(np.float32)[_np(z).astype(np.int64)]
    h = np.concatenate([h, _np(x).astype(np.float32)], -1)      # [12000, 19]

    zz_l = build_zz(h, pos, _np(edge_index), _np(edge_attr)[:, :4], 480, 25)
    wk0, wk1, biases = pack_weights(p["local_blocks"], p["local_fc1"],
                                    p["local_fc2"], 25, 16)
    zl = run_stage("local", LOCAL_CFG, zz_l, wk0, wk1, biases)  # [480, 16]

    zz_g = build_zz(zl, original_pos, _np(original_edge_index),
                    _np(original_edge_attr)[:, :4], 16, 30)
    wk0g, wk1g, biasg = pack_weights(p["global_blocks"], p["global_fc1"],
                                     p["global_fc2"], 22, 1)
    zg = run_stage("global", GLOBAL_CFG, zz_g, wk0g, wk1g, biasg)  # [16, 1]
    return np.ascontiguousarray(zg.reshape(-1)).astype(np.float32)
